# revision 5
# baseline (speedup 1.0000x reference)
"""MiniBatchDiscrimination kernel, v4: symmetric-pair sharding +
column-tiled paired PE reduction.

Math per core (row block of 64 i's x FD=320 j columns spanning 5 blocks):
  Mt[(o,k), j] = M^T in bf16 (16 partition-tiles), computed on PE.
  For each i:
    relu tiles (DVE, 4x bf16 tensor_scalar):  R_t = max(Mt_t - Mt_t[:,i], 0)
    abs tiles  (ACT offload, 2 tiles):        A_t = |Mt_t - Mt_t[:,i]|
    D[o,j] = sum_A |d| + 2*sum_R relu(d) - (S_j - S_i),  S = sum_k Mt (relu
    tiles only), so D = fold(psumA + psumB) with S_i applied as the Exp bias.
  The 16+1 reduction matmuls are issued as 8 column-tiled PAIRS: tile 2p ->
  PSUM partitions 0:64 (col group 0-1), tile 2p+1 -> partitions 64:128 (col
  group 2-3, tile_position=(0,64)).  The two streams run concurrently on
  disjoint array column groups, nearly halving PE time.  A DVE tensor_add
  folds the halves into a fresh bf16 tile; Exp(scale=-1, bias=-S_i,
  accum_out=rowsum) reads the fold.  The fold is software-pipelined one
  iteration behind the matmuls so its PE wait never stalls the DVE - and it
  doubles as DVE's PE-clock refresh, so slot-reuse waits are pre-observed
  (the walrus here encodes at most ONE sync wait per instruction).
  Column-sum partials (for the partner blocks, by symmetry) are reduced on
  PE from the packed exp tiles once per 8 rows and accumulated in fp32.
"""

import numpy as np
import ml_dtypes
from contextlib import ExitStack

BATCH, IN_FEAT, OUT_FEAT, KERNEL_DIM = 512, 512, 64, 32
N_CORES = 8
ROWB = BATCH // N_CORES          # 64 rows of i per core
OK = OUT_FEAT * KERNEL_DIM       # 2048 flattened (o,k)
NT = OK // 128                   # 16 partition-tiles of (o,k)
NBLK = 5                         # column blocks per core
FD = NBLK * 64                   # 320
POISON = 1.0e4

CHUNK = 8                        # i's per colsum PSUM chunk
SELW = OUT_FEAT
ACT_TILES = (5, 7, 11)           # elementwise tiles computed on ACT as Abs
ADV_BUFS = 56                    # 4 iterations of DVE elementwise tiles

_cache = {}


def _build_nc(split_waits=True):
    import concourse.bass as bass
    import concourse.mybir as mybir
    import concourse.tile as tile

    dt = mybir.dt
    AF = mybir.ActivationFunctionType
    OP = mybir.AluOpType

    nc = bass.Bass("TRN2", target_bir_lowering=False, debug=False,
                   num_devices=N_CORES)

    xT_d = nc.dram_tensor("xT", [IN_FEAT, FD], dt.bfloat16, kind="ExternalInput")
    T_d = nc.dram_tensor("Tm", [IN_FEAT, OK], dt.bfloat16, kind="ExternalInput")
    sel_d = nc.dram_tensor("sel", [128, NT * SELW], dt.bfloat16,
                           kind="ExternalInput")
    sel2_d = nc.dram_tensor("sel2", [128, OUT_FEAT], dt.bfloat16,
                            kind="ExternalInput")
    selS_d = nc.dram_tensor("selS", [128, NT * SELW], dt.bfloat16,
                            kind="ExternalInput")
    dneg_d = nc.dram_tensor("dneg", [OUT_FEAT, OUT_FEAT], dt.bfloat16,
                            kind="ExternalInput")
    rows_d = nc.dram_tensor("rowS", [OUT_FEAT, ROWB], dt.float32,
                            kind="ExternalOutput")
    acc_d = nc.dram_tensor("accS", [OUT_FEAT, FD], dt.float32,
                           kind="ExternalOutput")

    with tile.TileContext(nc) as tc, ExitStack() as ctx:
        const = ctx.enter_context(tc.tile_pool(name="const", bufs=1))
        mtp = ctx.enter_context(tc.tile_pool(name="mt", bufs=NT))
        psA = ctx.enter_context(
            tc.tile_pool(name="psA", bufs=1, space=bass.MemorySpace.PSUM))
        psDA = ctx.enter_context(
            tc.tile_pool(name="psDA", bufs=2, space=bass.MemorySpace.PSUM))
        psDB = ctx.enter_context(
            tc.tile_pool(name="psDB", bufs=2, space=bass.MemorySpace.PSUM))
        psC = ctx.enter_context(
            tc.tile_pool(name="psC", bufs=2, space=bass.MemorySpace.PSUM))
        workV = ctx.enter_context(tc.tile_pool(name="workV", bufs=ADV_BUFS))
        # ACT-written tiles (Abs elementwise + Exp outputs) share one pool:
        # the WAW chain keeps ACT's scheduled order near program order
        ep = ctx.enter_context(tc.tile_pool(name="e", bufs=48))
        # fold outputs are fresh (never reused) so the fold and the Exp that
        # reads it each carry exactly one wait
        foldp = ctx.enter_context(tc.tile_pool(name="fold", bufs=ROWB))

        Tsb = []
        for kc in range(4):
            t_ = const.tile([128, OK], dt.bfloat16, tag=f"T{kc}")
            nc.sync.dma_start(t_[:], T_d[kc * 128:(kc + 1) * 128, :])
            Tsb.append(t_)
        xTsb = []
        for kc in range(4):
            t_ = const.tile([128, FD], dt.bfloat16, tag=f"x{kc}")
            nc.sync.dma_start(t_[:], xT_d[kc * 128:(kc + 1) * 128, :])
            xTsb.append(t_)
        sel = const.tile([128, NT * SELW], dt.bfloat16, tag="sel")
        nc.sync.dma_start(sel[:], sel_d[:])
        sel2 = const.tile([128, OUT_FEAT], dt.bfloat16, tag="sel2")
        nc.sync.dma_start(sel2[:], sel2_d[:])
        selS = const.tile([128, NT * SELW], dt.bfloat16, tag="selS")
        nc.sync.dma_start(selS[:], selS_d[:])
        dneg = const.tile([OUT_FEAT, OUT_FEAT], dt.bfloat16, tag="dneg")
        nc.sync.dma_start(dneg[:], dneg_d[:])
        mcol = const.tile([128, NT * ROWB], dt.float32, tag="mcol")
        rowS = const.tile([OUT_FEAT, ROWB], dt.float32, tag="rowS")
        accS = const.tile([OUT_FEAT, FD], dt.float32, tag="accS")
        nc.vector.memset(accS[:], 0.0)

        # Mt tiles: Mt[(o,k), j], tile t holds o in [4t, 4t+4), all k
        mts = []
        for t in range(NT):
            ps = psA.tile([128, FD], dt.float32)
            for kc in range(4):
                nc.tensor.matmul(ps[:],
                                 Tsb[kc][:, t * 128:(t + 1) * 128],
                                 xTsb[kc][:],
                                 start=(kc == 0), stop=(kc == 3))
            mt_t = mtp.tile([128, FD], dt.bfloat16, tag="mt")
            nc.vector.tensor_copy(mt_t[:], ps[:])
            # scalar columns: the *rounded* bf16 values recast to fp32 so the
            # diagonal difference is exactly zero
            nc.vector.tensor_copy(mcol[:, t * ROWB:(t + 1) * ROWB],
                                  mt_t[:, 0:ROWB])
            mts.append(mt_t)

        # S[o, j] = sum_k Mt[(o,k), j] over the relu tiles only; kept in
        # bf16 so the Exp bias cancels the matmul term exactly on the
        # diagonal: D_ii = 2*0 + S_i - S_i = 0.
        r_tiles = [t for t in range(NT) if t not in ACT_TILES]
        psS = psA.tile([OUT_FEAT, FD], dt.float32, tag="psS")
        for m, t in enumerate(r_tiles):
            nc.tensor.matmul(psS[:], selS[:, t * SELW:(t + 1) * SELW],
                             mts[t][:], start=(m == 0),
                             stop=(m == len(r_tiles) - 1))
        S_bf = const.tile([OUT_FEAT, FD], dt.bfloat16, tag="S_bf")
        nc.vector.tensor_copy(S_bf[:], psS[:])
        Sneg = const.tile([OUT_FEAT, ROWB], dt.float32, tag="Sneg")
        nc.vector.tensor_scalar(Sneg[:], S_bf[:, 0:ROWB], -1.0, None,
                                op0=OP.mult)
        # warm up ACT's observed DVE clock so the first ACT op (reading
        # DVE-written tiles) does not need a second sync wait
        warmA = const.tile([1, 1], dt.float32, tag="warmA")
        nc.scalar.copy(warmA[:], Sneg[0:1, 0:1])

        e_tiles_of = {}          # chunk -> list of packed e tiles
        pending = None           # (psd2, i) awaiting fold+exp

        def fold_exp(pair, i):
            psda, psdb = pair
            # move the B half to SBUF (bf16) and fold it into the A bank on
            # PE via an identity matmul (sel2's top half is I64); the copy
            # doubles as DVE's PE-clock refresh
            b_sb = foldp.tile([OUT_FEAT, FD], dt.bfloat16, tag="fold",
                              name=f"bsb_{i}")
            nc.vector.tensor_copy(b_sb[:], psdb[OUT_FEAT:128, :])
            nc.tensor.matmul(psda[:], sel2[0:OUT_FEAT, :],
                             b_sb[:], start=False, stop=True)
            ch = i // CHUNK
            if i % 2 == 0:
                e_t = ep.tile([128, FD], dt.bfloat16, tag="e",
                              name=f"e_{i}")
                e_tiles_of.setdefault(ch, []).append(e_t)
            half = e_tiles_of[ch][-1][(i % 2) * OUT_FEAT:
                                      (i % 2 + 1) * OUT_FEAT, :]
            nc.scalar.activation(half, psda[:], AF.Exp,
                                 scale=-1.0, bias=Sneg[:, i:i + 1],
                                 accum_out=rowS[:, i:i + 1])
            if i % CHUNK == CHUNK - 1:
                # column-sum partials for this chunk
                psc = psC.tile([OUT_FEAT, FD], dt.float32)
                ets = e_tiles_of[ch]
                for m, e_t in enumerate(ets):
                    nc.tensor.matmul(psc[:], sel2[:], e_t[:],
                                     start=(m == 0),
                                     stop=(m == len(ets) - 1))
                nc.vector.tensor_add(accS[:], accS[:], psc[:])

        for i in range(ROWB):
            psda = psDA.tile([OUT_FEAT, FD], dt.float32, tag="psda",
                             name=f"psda_{i}")
            psdb = psDB.tile([128, FD], dt.float32, tag="psdb",
                             name=f"psdb_{i}")
            ads = {}
            for t in range(NT):
                sc = mcol[:, t * ROWB + i: t * ROWB + i + 1]
                if t in ACT_TILES:
                    ad_t = ep.tile([128, FD], dt.bfloat16, tag="e",
                                   name=f"adA_{i}_{t}")
                    nc.scalar.activation(ad_t[:], mts[t][:], AF.Abs,
                                         bias=sc, scale=-1.0)
                else:
                    ad_t = workV.tile([128, FD], dt.bfloat16, tag="adV",
                                      name=f"ad_{i}_{t}")
                    nc.vector.tensor_scalar(ad_t[:], mts[t][:], sc, 0.0,
                                            op0=OP.subtract, op1=OP.max)
                ads[t] = ad_t
            # 8 column-tiled matmul pairs: even tile -> partitions 0:64
            # (array col group 0-1), odd tile -> 64:128 (col group 2-3);
            # the two streams use disjoint column groups and overlap
            for p in range(NT // 2):
                nc.tensor.matmul(psda[:],
                                 sel[:, (2 * p) * SELW:(2 * p + 1) * SELW],
                                 ads[2 * p][:],
                                 start=(p == 0), stop=False)
                nc.tensor.matmul(psdb[OUT_FEAT:128, :],
                                 sel[:, (2 * p + 1) * SELW:
                                      (2 * p + 2) * SELW],
                                 ads[2 * p + 1][:],
                                 start=(p == 0), stop=(p == NT // 2 - 1),
                                 tile_position=(0, 64))
            # -S_j correction joins the A bank (group stays open: the
            # fold matmul emitted next iteration closes it)
            nc.tensor.matmul(psda[:], dneg[:], S_bf[:],
                             start=False, stop=False)
            # fold+exp of the PREVIOUS iteration: its PE wait is already
            # satisfied, so the DVE never stalls, and it refreshes DVE's
            # observed PE clock for the elementwise slot reuse
            if pending is not None:
                fold_exp(*pending)
            pending = ((psda, psdb), i)
        fold_exp(*pending)

        # outputs go out on the SW-DGE queues (gpsimd): the HW-DGE queues
        # carried the input loads, and a shared queue would add a second
        # sync-wait command that the DMA pseudo-instruction cannot encode
        nc.gpsimd.dma_start(rows_d[:], rowS[:])
        nc.gpsimd.dma_start(acc_d[:], accS[:])

    if split_waits:
        _split_multiwaits(nc, mybir)
    return nc


def _split_multiwaits(nc, mybir):
    """Walrus on this toolchain encodes at most ONE sync-wait command per
    instruction.  Split any instruction with more waits (in practice only
    the framework's kernel-tail drain) into a chain of single-wait Drain
    carriers on the same engine, inserted immediately before it."""
    n = 0
    for fn in nc.m.functions:
        for bb in fn.blocks:
            new_insts = []
            for inst in bb.instructions:
                si = getattr(inst, "sync_info", None)
                if si is not None and si.on_wait and len(si.on_wait) > 1:
                    waits = list(si.on_wait)
                    for w in waits[:-1]:
                        carrier = mybir.InstDrain(
                            name=f"splitw_{n}", engine=inst.engine,
                            ins=[], outs=[],
                            sync_info=mybir.SyncInfo(on_wait=[w],
                                                     on_update=[]))
                        new_insts.append(carrier)
                        n += 1
                    inst.sync_info = mybir.SyncInfo(
                        on_wait=[waits[-1]], on_update=list(si.on_update))
                new_insts.append(inst)
            if n:
                bb.instructions = new_insts


def _sel_host(value, act_value=None):
    sel = np.zeros((128, NT * SELW), dtype=np.float32)
    for t in range(NT):
        v = value if (act_value is None or t not in ACT_TILES) else act_value
        for g in range(4):
            sel[32 * g:32 * (g + 1), t * SELW + 4 * t + g] = v
    return sel.astype(ml_dtypes.bfloat16)


def _sel2_host():
    s = np.zeros((128, OUT_FEAT), dtype=np.float32)
    s[:OUT_FEAT, :] = np.eye(OUT_FEAT)
    s[OUT_FEAT:, :] = np.eye(OUT_FEAT)
    return s.astype(ml_dtypes.bfloat16)


def _block_order(c):
    """Column blocks for core c; None marks the poison block."""
    if c < 4:
        return [c, c + 1, c + 2, c + 3, c + 4]
    return [c, (c + 1) % 8, (c + 2) % 8, (c + 3) % 8, None]


def _in_maps(x, T):
    bf16 = ml_dtypes.bfloat16
    Tb = np.ascontiguousarray(T.reshape(IN_FEAT, OK)).astype(bf16)
    selb = _sel_host(2.0, act_value=1.0)
    selSb = _sel_host(1.0)
    sel2b = _sel2_host()
    dnegb = (-np.eye(OUT_FEAT, dtype=np.float32)).astype(bf16)
    xT = np.ascontiguousarray(x.T)
    maps = []
    for c in range(N_CORES):
        xTc = np.empty((IN_FEAT, FD), dtype=np.float32)
        for pos, b in enumerate(_block_order(c)):
            if b is None:
                xTc[:, 64 * pos:64 * (pos + 1)] = POISON
            else:
                xTc[:, 64 * pos:64 * (pos + 1)] = xT[:, 64 * b:64 * (b + 1)]
        maps.append({"xT": xTc.astype(bf16), "Tm": Tb, "sel": selb,
                     "selS": selSb, "sel2": sel2b, "dneg": dnegb})
    return maps


def kernel(x, T):
    from concourse import bass_utils

    x = np.asarray(x, dtype=np.float32)
    T = np.asarray(T, dtype=np.float32)

    if "nc" not in _cache:
        _cache["nc"] = _build_nc()
    nc = _cache["nc"]

    res = bass_utils.run_bass_kernel_spmd(
        nc, _in_maps(x, T), core_ids=list(range(N_CORES)))

    mbd = np.zeros((BATCH, OUT_FEAT), dtype=np.float32)
    for c in range(N_CORES):
        rs = np.asarray(res.results[c]["rowS"], dtype=np.float32)  # [o, i]
        mbd[64 * c:64 * (c + 1), :] += rs.T
        acc = np.asarray(res.results[c]["accS"], dtype=np.float32)  # [o, j]
        for pos, b in enumerate(_block_order(c)):
            if pos == 0 or b is None:
                continue  # own diag block is fully in rowsums; poison dropped
            mbd[64 * b:64 * (b + 1), :] += acc[:, 64 * pos:64 * (pos + 1)].T
    mbd -= 1.0
    return np.concatenate([x, mbd], axis=1)
